# revision 4
# baseline (speedup 1.0000x reference)
import numpy as np

B = 8
SEQ = 4096
D = 1024
N_BASE = 10000.0
N_CORES = 8
SPC = SEQ // N_CORES   # seq rows per core (512)
H = 512                # f32 per unit (half row)
UPC = SPC * D // H     # units per core (1024)

# SDMA engine 15 (partitions 92-95 & 124-127, port = ((p>>2)&7)<<1 | (p>>6))
# runs ~17% slower than engines 0-14. Skew the per-partition load so its
# partitions carry 7 units where the rest carry 8; the displaced units
# become an 8th unit on the 120 fast partitions plus a 9th on partitions
# 24-31 (one per even port). Unit u of a chunk maps to slots in DRAM
# order, so DRAM keeps the natural row-major layout.
NA = 7 * 128           # block A units: j<7 on all partitions (896)
NB1 = 92               # block B1: j=7 on partitions 0-91
NB2 = 28               # block B2: j=7 on partitions 96-123
NC9 = 8                # block C: j=8 on partitions 24-31
assert NA + NB1 + NB2 + NC9 == UPC

_CACHE = {}


def _compute_pe() -> np.ndarray:
    """Mirror of the reference _pos_encoding (default jax backend, f32)."""
    import jax
    import jax.numpy as jnp

    pos = jnp.arange(SEQ, dtype=jnp.float32)[:, None]
    i = jnp.arange(D // 2, dtype=jnp.float32)
    denom = jnp.power(jnp.float32(N_BASE), 2.0 * i / jnp.float32(D))
    ang = pos / denom
    pe = jnp.stack([jnp.sin(ang), jnp.cos(ang)], axis=-1).reshape(SEQ, D)
    return np.asarray(jax.device_get(pe), dtype=np.float32)


def _skew_dmas(nc, engine, dram, row0, tile, to_sbuf):
    """The 4 rectangle DMAs moving one chunk (UPC units at dram[row0:])
    between DRAM (natural order) and the skewed SBUF tile layout."""
    a = dram[row0 : row0 + NA, :].rearrange("(p j) d -> p j d", j=7)
    b1 = dram[row0 + NA : row0 + NA + NB1, :].rearrange(
        "(p j) d -> p j d", j=1
    )
    b2 = dram[row0 + NA + NB1 : row0 + NA + NB1 + NB2, :].rearrange(
        "(p j) d -> p j d", j=1
    )
    c = dram[row0 + UPC - NC9 : row0 + UPC, :].rearrange(
        "(p j) d -> p j d", j=1
    )
    pairs = [
        (tile[:, 0:7, :], a),
        (tile[0:92, 7:8, :], b1),
        (tile[96:124, 7:8, :], b2),
        (tile[24:32, 8:9, :], c),
    ]
    for sb, dr in pairs:
        if to_sbuf:
            engine.dma_start(out=sb, in_=dr)
        else:
            engine.dma_start(out=dr, in_=sb)


def _build_program():
    import concourse.bacc as bacc
    import concourse.mybir as mybir
    import concourse.tile as tile

    nc = bacc.Bacc("TRN2")
    f32 = mybir.dt.float32
    x_in = nc.declare_dram_parameter("x", [B * UPC, H], f32, isOutput=False)
    pe_in = nc.declare_dram_parameter("pe", [UPC, H], f32, isOutput=False)
    y_out = nc.declare_dram_parameter("y", [B * UPC, H], f32, isOutput=True)

    with tile.TileContext(nc) as tc:
        with (
            tc.tile_pool(name="pe_pool", bufs=1) as pe_pool,
            tc.tile_pool(name="x_pool", bufs=B) as x_pool,
        ):
            pe_t = pe_pool.tile([128, 9, H], f32)
            # pe rides the (initially idle) scalar/output queue so the
            # x stream starts on the sync queue immediately.
            _skew_dmas(nc, nc.scalar, pe_in, 0, pe_t, to_sbuf=True)
            xts = []
            for b in range(B):
                xt = x_pool.tile([128, 9, H], f32)
                _skew_dmas(nc, nc.sync, x_in, b * UPC, xt, to_sbuf=True)
                xts.append(xt)
            for b in range(B):
                nc.vector.tensor_add(xts[b][:], xts[b][:], pe_t[:])
                _skew_dmas(nc, nc.scalar, y_out, b * UPC, xts[b], to_sbuf=False)
    if not nc.is_finalized():
        nc.finalize()
    return nc


def _get_state():
    if "nc" not in _CACHE:
        _CACHE["nc"] = _build_program()
    if "pe" not in _CACHE:
        _CACHE["pe"] = _compute_pe()
    return _CACHE["nc"], _CACHE["pe"]


def _in_maps(x, pe):
    in_maps = []
    for c in range(N_CORES):
        xs = np.ascontiguousarray(x[:, c * SPC : (c + 1) * SPC, :]).reshape(
            B * UPC, H
        )
        pes = np.ascontiguousarray(pe[c * SPC : (c + 1) * SPC, :]).reshape(
            UPC, H
        )
        in_maps.append({"x": xs, "pe": pes})
    return in_maps


def kernel(x, seq_len=None, **_):
    from concourse.bass_utils import run_bass_kernel_spmd

    x = np.asarray(x, dtype=np.float32)
    assert x.shape == (B, SEQ, D)
    if seq_len is not None:
        assert int(np.asarray(seq_len)) == SEQ

    nc, pe = _get_state()
    res = run_bass_kernel_spmd(nc, _in_maps(x, pe), list(range(N_CORES))).results

    out = np.empty((B, SEQ, D), dtype=np.float32)
    for c in range(N_CORES):
        out[:, c * SPC : (c + 1) * SPC, :] = res[c]["y"].reshape(B, SPC, D)
    return out


# revision 6
# speedup vs baseline: 1.2113x; 1.2113x over previous
import numpy as np

B = 8
SEQ = 4096
D = 1024
N_BASE = 10000.0
N_CORES = 8
SPC = SEQ // N_CORES   # seq rows per core (512)
H = 128                # f32 per 512B unit
UPC = SPC * D // H     # 512B units per core chunk (4096)

# SDMA engine 15 is ~17% slower than engines 0-14 (persistent hardware
# trait; HWDGE splits a DMA's descriptor list into equal runs of
# g = smallest divisor of n >= n/16, assigned to engines 0..n/g-1).
# Split each 2MB chunk into three DMAs so engine 15 carries ~86% of the
# per-engine load while every partition still holds exactly 32 units:
#   M: [128 parts] x 28 units (14KB descs) -> engines 0-15, 8 descs each
#   A: [120 parts] x 4 units  (2KB descs)  -> engines 0-14, 8 descs each
#   T: [8 parts]   x 4 units  (2KB descs)  -> engines 0-7, 1 desc each
UB = 28                # units per partition in the M block
UA = 4                 # units per partition in A (parts 0-119) / T (120-127)
NM = 128 * UB          # 3584 units
NA = 120 * UA          # 480 units
NT = 8 * UA            # 32 units
assert NM + NA + NT == UPC

_CACHE = {}


def _compute_pe() -> np.ndarray:
    """Mirror of the reference _pos_encoding (default jax backend, f32)."""
    import jax
    import jax.numpy as jnp

    pos = jnp.arange(SEQ, dtype=jnp.float32)[:, None]
    i = jnp.arange(D // 2, dtype=jnp.float32)
    denom = jnp.power(jnp.float32(N_BASE), 2.0 * i / jnp.float32(D))
    ang = pos / denom
    pe = jnp.stack([jnp.sin(ang), jnp.cos(ang)], axis=-1).reshape(SEQ, D)
    return np.asarray(jax.device_get(pe), dtype=np.float32)


def _skew_dmas(nc, engine, dram, row0, tile, to_sbuf):
    """Move one chunk (UPC units at dram[row0:]) between DRAM (natural
    order) and an SBUF tile [128, 32, H] via the M/A/T split."""
    m = dram[row0 : row0 + NM, :].rearrange("(p j) d -> p j d", j=UB)
    a = dram[row0 + NM : row0 + NM + NA, :].rearrange(
        "(p j) d -> p j d", j=UA
    )
    t = dram[row0 + NM + NA : row0 + UPC, :].rearrange(
        "(p j) d -> p j d", j=UA
    )
    pairs = [
        (tile[:, 0:UB, :], m),
        (tile[0:120, UB : UB + UA, :], a),
        (tile[120:128, UB : UB + UA, :], t),
    ]
    for sb, dr in pairs:
        if to_sbuf:
            engine.dma_start(out=sb, in_=dr)
        else:
            engine.dma_start(out=dr, in_=sb)


def _build_program():
    import concourse.bacc as bacc
    import concourse.mybir as mybir
    import concourse.tile as tile

    nc = bacc.Bacc("TRN2")
    f32 = mybir.dt.float32
    x_in = nc.declare_dram_parameter("x", [B * UPC, H], f32, isOutput=False)
    pe_in = nc.declare_dram_parameter("pe", [UPC, H], f32, isOutput=False)
    y_out = nc.declare_dram_parameter("y", [B * UPC, H], f32, isOutput=True)

    with tile.TileContext(nc) as tc:
        with (
            tc.tile_pool(name="pe_pool", bufs=1) as pe_pool,
            tc.tile_pool(name="x_pool", bufs=B) as x_pool,
        ):
            pe_t = pe_pool.tile([128, UB + UA, H], f32)
            # pe rides the (initially idle) scalar/output queue so the
            # x stream starts on the sync queue immediately.
            _skew_dmas(nc, nc.scalar, pe_in, 0, pe_t, to_sbuf=True)
            xts = []
            for b in range(B):
                xt = x_pool.tile([128, UB + UA, H], f32)
                _skew_dmas(nc, nc.sync, x_in, b * UPC, xt, to_sbuf=True)
                xts.append(xt)
            for b in range(B):
                nc.vector.tensor_add(xts[b][:], xts[b][:], pe_t[:])
                _skew_dmas(nc, nc.scalar, y_out, b * UPC, xts[b], to_sbuf=False)
    if not nc.is_finalized():
        nc.finalize()
    return nc


def _get_state():
    if "nc" not in _CACHE:
        _CACHE["nc"] = _build_program()
    if "pe" not in _CACHE:
        _CACHE["pe"] = _compute_pe()
    return _CACHE["nc"], _CACHE["pe"]


def _in_maps(x, pe):
    in_maps = []
    for c in range(N_CORES):
        xs = np.ascontiguousarray(x[:, c * SPC : (c + 1) * SPC, :]).reshape(
            B * UPC, H
        )
        pes = np.ascontiguousarray(pe[c * SPC : (c + 1) * SPC, :]).reshape(
            UPC, H
        )
        in_maps.append({"x": xs, "pe": pes})
    return in_maps


def kernel(x, seq_len=None, **_):
    from concourse.bass_utils import run_bass_kernel_spmd

    x = np.asarray(x, dtype=np.float32)
    assert x.shape == (B, SEQ, D)
    if seq_len is not None:
        assert int(np.asarray(seq_len)) == SEQ

    nc, pe = _get_state()
    res = run_bass_kernel_spmd(nc, _in_maps(x, pe), list(range(N_CORES))).results

    out = np.empty((B, SEQ, D), dtype=np.float32)
    for c in range(N_CORES):
        out[:, c * SPC : (c + 1) * SPC, :] = res[c]["y"].reshape(B, SPC, D)
    return out


# revision 10
# speedup vs baseline: 1.2204x; 1.0075x over previous
import numpy as np

B = 8
SEQ = 4096
D = 1024
N_BASE = 10000.0
N_CORES = 8
SPC = SEQ // N_CORES   # seq rows per core (512)
H = 128                # f32 per 512B unit
UPP = 32               # units per partition per chunk (16KB)
UPC = SPC * D // H     # units per core chunk (4096)

_CACHE = {}


def _compute_pe() -> np.ndarray:
    """Mirror of the reference _pos_encoding (default jax backend, f32)."""
    import jax
    import jax.numpy as jnp

    pos = jnp.arange(SEQ, dtype=jnp.float32)[:, None]
    i = jnp.arange(D // 2, dtype=jnp.float32)
    denom = jnp.power(jnp.float32(N_BASE), 2.0 * i / jnp.float32(D))
    ang = pos / denom
    pe = jnp.stack([jnp.sin(ang), jnp.cos(ang)], axis=-1).reshape(SEQ, D)
    return np.asarray(jax.device_get(pe), dtype=np.float32)


def _pass_dmas(nc, engine, dram, row0, tile, u0, nu, to_sbuf, skip15):
    """Move [128 parts x nu units] between the chunk at dram[row0:]
    (natural order: partition p holds units [UPP*p, UPP*p+UPP)) and
    tile[:, u0:u0+nu, :], for the unit column range [u0, u0+nu).

    skip15=False: one [128]-DMA -> 16 engines x 8 descs (uniform).
    skip15=True: a [120]-DMA (engines 0-14) + an [8]-DMA (engines 0-7),
    so SDMA engine 15 (~17% slower than 0-14) gets nothing. HWDGE splits
    a DMA's n descriptors into runs of g = smallest divisor of n that is
    >= n/16, assigned to engines 0..n/g-1.
    """
    view = dram[row0 : row0 + 128 * UPP, :].rearrange(
        "(p j) d -> p j d", j=UPP
    )
    us = slice(u0, u0 + nu)
    if not skip15:
        pairs = [(tile[:, us, :], view[:, us, :])]
    else:
        pairs = [
            (tile[0:120, us, :], view[0:120, us, :]),
            (tile[120:128, us, :], view[120:128, us, :]),
        ]
    for sb, dr in pairs:
        if to_sbuf:
            engine.dma_start(out=sb, in_=dr)
        else:
            engine.dma_start(out=dr, in_=sb)


def _build_program():
    import concourse.bacc as bacc
    import concourse.mybir as mybir
    import concourse.tile as tile

    nc = bacc.Bacc("TRN2")
    f32 = mybir.dt.float32
    x_in = nc.declare_dram_parameter("x", [B * UPC, H], f32, isOutput=False)
    pe_in = nc.declare_dram_parameter("pe", [UPC, H], f32, isOutput=False)
    y_out = nc.declare_dram_parameter("y", [B * UPC, H], f32, isOutput=True)

    with tile.TileContext(nc) as tc:
        with (
            tc.tile_pool(name="pe_pool", bufs=1) as pe_pool,
            tc.tile_pool(name="x_pool", bufs=B) as x_pool,
        ):
            pe_t = pe_pool.tile([128, UPP, H], f32)
            # pe rides the (initially idle) scalar/output queue, skipping
            # engine 15 so the slow engine only carries uniform passes.
            _pass_dmas(nc, nc.scalar, pe_in, 0, pe_t, 0, UPP, True, True)
            xts = []
            for b in range(B):
                xt = x_pool.tile([128, UPP, H], f32)
                if b < B - 1:
                    _pass_dmas(
                        nc, nc.sync, x_in, b * UPC, xt, 0, UPP, True, False
                    )
                else:
                    # last chunk: two half-passes, engine-15-free, so the
                    # final input lands fast and the last add is short
                    hu = UPP // 2
                    _pass_dmas(
                        nc, nc.sync, x_in, b * UPC, xt, 0, hu, True, True
                    )
                    _pass_dmas(
                        nc, nc.sync, x_in, b * UPC, xt, hu, hu, True, True
                    )
                xts.append(xt)
            for b in range(B - 1):
                nc.vector.tensor_add(xts[b][:], xts[b][:], pe_t[:])
                _pass_dmas(
                    nc, nc.scalar, y_out, b * UPC, xts[b], 0, UPP, False, False
                )
            # last chunk: two half adds + engine-15-free half writes
            b = B - 1
            hu = UPP // 2
            for hi in range(2):
                sl = slice(hi * hu, (hi + 1) * hu)
                nc.vector.tensor_add(
                    xts[b][:, sl, :], xts[b][:, sl, :], pe_t[:, sl, :]
                )
                _pass_dmas(
                    nc,
                    nc.scalar,
                    y_out,
                    b * UPC,
                    xts[b],
                    hi * hu,
                    hu,
                    False,
                    True,
                )
    if not nc.is_finalized():
        nc.finalize()
    return nc


def _get_state():
    if "nc" not in _CACHE:
        _CACHE["nc"] = _build_program()
    if "pe" not in _CACHE:
        _CACHE["pe"] = _compute_pe()
    return _CACHE["nc"], _CACHE["pe"]


def _in_maps(x, pe):
    in_maps = []
    for c in range(N_CORES):
        xs = np.ascontiguousarray(x[:, c * SPC : (c + 1) * SPC, :]).reshape(
            B * UPC, H
        )
        pes = np.ascontiguousarray(pe[c * SPC : (c + 1) * SPC, :]).reshape(
            UPC, H
        )
        in_maps.append({"x": xs, "pe": pes})
    return in_maps


def kernel(x, seq_len=None, **_):
    from concourse.bass_utils import run_bass_kernel_spmd

    x = np.asarray(x, dtype=np.float32)
    assert x.shape == (B, SEQ, D)
    if seq_len is not None:
        assert int(np.asarray(seq_len)) == SEQ

    nc, pe = _get_state()
    res = run_bass_kernel_spmd(nc, _in_maps(x, pe), list(range(N_CORES))).results

    out = np.empty((B, SEQ, D), dtype=np.float32)
    for c in range(N_CORES):
        out[:, c * SPC : (c + 1) * SPC, :] = res[c]["y"].reshape(B, SPC, D)
    return out
